# revision 6
# baseline (speedup 1.0000x reference)
"""GNN mean-aggregation + 2-layer MLP on 8 TRN2 cores, v5.

v3 + per-group-K slot packing (drops superblock max-K padding, ~408k vs
~433k gathered slots/core), combine split into two halves so the MLP
overlaps the second half's combine gathers, deeper gather/idx buffering.
Pipeline remains bound by the serial Q7 SWDGE descriptor-generation rate
(~7.3ns per gathered slot).
"""

import math

import numpy as np

N_CORES = 8
P = 128
GROUP = N_CORES * P
FH = 16
F = 32
HID = 128
BLOCK_GROUPS = 4
SLOT_CAP = 63  # max dst slot-columns per gather instr (8064 idxs = 505 descs)


# ---------------------------------------------------------------- host prep


def _host_prep(x, edge_index):
    N = x.shape[0]
    CHS = N // 4
    i = edge_index[0].astype(np.int64)
    j = edge_index[1].astype(np.int64)
    rows = np.concatenate([i, j])
    cols = np.concatenate([j, i])
    deg = np.bincount(rows, minlength=N)

    order = np.argsort(-deg, kind="stable")  # new id -> orig id
    newid = np.empty(N, np.int64)
    newid[order] = np.arange(N)

    n_groups = math.ceil(N / GROUP)
    NLOC = n_groups * P

    NT = 4 * (CHS + 1)
    table = np.zeros((NT, 64), np.float32)
    for q in range(4):
        table[q * (CHS + 1) : q * (CHS + 1) + CHS, :FH] = x[
            q * CHS : (q + 1) * CHS, FH:F
        ]

    nrow = newid[rows]
    core_of = (nrow % GROUP) // P
    loc_of = (nrow // GROUP) * P + (nrow % P)
    q_of = cols // CHS
    local_src = (cols - q_of * CHS).astype(np.int64)

    cnt = np.zeros((N_CORES, NLOC, 4), np.int32)
    np.add.at(cnt, (core_of, loc_of, q_of), 1)

    ekey = (core_of * 4 + q_of) * NLOC + loc_of
    eord = np.argsort(ekey, kind="stable")
    svals = local_src[eord]
    counts_flat = np.zeros(N_CORES * 4 * NLOC, np.int64)
    np.add.at(counts_flat, ekey, 1)
    ptr = np.zeros(N_CORES * 4 * NLOC + 1, np.int64)
    ptr[1:] = np.cumsum(counts_flat)

    perm = np.zeros((N_CORES, 4, NLOC), np.int64)
    csort = np.zeros((N_CORES, 4, NLOC), np.int32)
    for c in range(N_CORES):
        for q in range(4):
            pm = np.argsort(-cnt[c, :, q], kind="stable")
            perm[c, q] = pm
            csort[c, q] = cnt[c, pm, q]
    Kq = np.zeros((4, n_groups), np.int64)
    for q in range(4):
        firsts = csort[:, q, ::P]
        Kq[q] = np.maximum(firsts.max(axis=0), 1)

    # batches: consecutive groups with per-group K, sum(K) <= SLOT_CAP
    batches = []  # (q, g0, [K_g...])
    for q in range(4):
        g0 = 0
        while g0 < n_groups:
            ks = [int(Kq[q][g0])]
            while g0 + len(ks) < n_groups and sum(ks) + int(
                Kq[q][g0 + len(ks)]
            ) <= SLOT_CAP:
                ks.append(int(Kq[q][g0 + len(ks)]))
            batches.append((q, g0, ks))
            g0 += len(ks)

    idx_cols = []
    idx_all = []
    for c in range(N_CORES):
        parts = []
        for bi, (q, g0, ks) in enumerate(batches):
            segs = []
            for gg, K in enumerate(ks):
                ranks = np.arange((g0 + gg) * P, (g0 + gg + 1) * P)
                locs = perm[c, q][ranks]
                cts = cnt[c, locs, q]
                base = ptr[(c * 4 + q) * NLOC + locs]
                kk = np.arange(K)
                pos = base[:, None] + kk[None, :]
                mask = kk[None, :] < cts[:, None]
                vals = np.where(mask, svals[np.minimum(pos, len(svals) - 1)], CHS)
                segs.append(vals.T.reshape(-1))  # k-major within group
            v3 = np.concatenate(segs)
            n_idx = len(v3)  # = 128*sum(ks)
            W = (n_idx + 15) // 16
            arr = np.zeros((P, W), np.int16)
            for pp in range(16):
                lane = v3[pp::16].astype(np.int16)
                arr[pp, : len(lane)] = lane
                arr[16 + pp, : len(lane)] = lane
            parts.append(arr)
            if c == 0:
                idx_cols.append(W)
        idx_all.append(np.concatenate(parts, axis=1))

    comb_idx = np.zeros((N_CORES, 4, P, (NLOC + 15) // 16), np.int16)
    for c in range(N_CORES):
        for q in range(4):
            invp = np.empty(NLOC, np.int64)
            invp[perm[c, q]] = np.arange(NLOC)
            for pp in range(16):
                lane = invp[pp::16].astype(np.int16)
                comb_idx[c, q, pp, : len(lane)] = lane
                comb_idx[c, q, 16 + pp, : len(lane)] = lane

    radial_all = np.zeros((N_CORES, FH, NLOC), np.float32)
    invdegw_all = np.ones((N_CORES, P, n_groups * FH), np.float32)
    nid = np.arange(N)
    c_arr = (nid % GROUP) // P
    l_arr = (nid // GROUP) * P + (nid % P)
    g_arr = l_arr // P
    p_arr = l_arr % P
    radial_all[c_arr, :, l_arr] = x[order, :FH]
    inv = (1.0 / np.maximum(deg[order], 1)).astype(np.float32)
    for f in range(FH):
        invdegw_all[c_arr, p_arr, g_arr * FH + f] = inv

    ident = np.eye(P, dtype=np.float32)
    return dict(
        order=order,
        n_groups=n_groups,
        NLOC=NLOC,
        NT=NT,
        batches=batches,
        idx_cols=idx_cols,
        idx_all=idx_all,
        comb_idx=comb_idx,
        table=table,
        radial_all=radial_all,
        invdegw_all=invdegw_all,
        ident=ident,
        CHS=CHS,
    )


# ------------------------------------------------------------- bass program


def _dma_gather_raw(gp, out_ap, in_ap, idxs_ap, num_idxs, elem_size, elem_step):
    from concourse import mybir

    dtsize = mybir.dt.size(in_ap.dtype)
    stride_bytes = elem_step * dtsize
    assert stride_bytes % 256 == 0 and stride_bytes // 256 < 256
    return gp.add_instruction(
        mybir.InstDMAGatherAnt(
            name=gp.bass.get_next_instruction_name(),
            ins=[
                *gp.lower_ap_dma(in_ap, for_custom_bir_dma=True),
                gp.lower_ap(idxs_ap),
                gp.lower_val_access(gp.to_reg(num_idxs)),
            ],
            outs=[gp.lower_ap(out_ap)],
            transpose=False,
            num_idxs=num_idxs,
            elem_size=elem_size,
            stride_bytes_256=stride_bytes // 256,
            gen_mode=0,
            single_packet=False,
            queue_num=0,
            sbuf_tokens_per_rank=0,
            sbuf_free_dim_per_rank=0,
            sbuf_free_dim_pad_per_rank=0,
            sbuf_byte_offset=0,
        )
    )


def build_program(prep):
    import concourse.tile as tile
    from concourse import bacc, mybir, library_config

    f32 = mybir.dt.float32
    i16 = mybir.dt.int16
    AF = mybir.ActivationFunctionType

    n_groups = prep["n_groups"]
    NLOC = prep["NLOC"]
    NT = prep["NT"]
    CHS = prep["CHS"]
    batches = prep["batches"]
    idx_cols = prep["idx_cols"]
    TOTW = sum(idx_cols)
    NW = (NLOC + 15) // 16
    nseg = min(4, n_groups)
    bseg, rseg = divmod(n_groups, nseg)
    segs_bc = [bseg + 1] * rseg + [bseg] * (nseg - rseg)

    nc = bacc.Bacc(None, dynamic_dma_scratch_size=98304)
    table = nc.dram_tensor("table", [NT, 64], f32, kind="ExternalInput")
    idxs = nc.dram_tensor("idxs", [P, TOTW], i16, kind="ExternalInput")
    cidx = nc.dram_tensor("cidx", [P, 4 * NW], i16, kind="ExternalInput")
    radial = nc.dram_tensor("radial", [FH, NLOC], f32, kind="ExternalInput")
    invdegw = nc.dram_tensor("invdegw", [P, n_groups * FH], f32, kind="ExternalInput")
    identt = nc.dram_tensor("identt", [P, P], f32, kind="ExternalInput")
    w1a = nc.dram_tensor("w1a", [FH, HID], f32, kind="ExternalInput")
    w1b = nc.dram_tensor("w1b", [FH, HID], f32, kind="ExternalInput")
    w2 = nc.dram_tensor("w2", [HID, F], f32, kind="ExternalInput")
    b1 = nc.dram_tensor("b1", [HID, 1], f32, kind="ExternalInput")
    b2 = nc.dram_tensor("b2", [1, F], f32, kind="ExternalInput")
    parts = [
        nc.dram_tensor(f"part{q}", [NLOC, 64], f32, kind="ExternalOutput")
        for q in range(4)
    ]
    out = nc.dram_tensor("out", [NLOC, F], f32, kind="ExternalOutput")

    with tile.TileContext(nc) as tc:
        with (
            tc.tile_pool(name="res", bufs=1) as res,
            tc.tile_pool(name="ixp", bufs=4) as ixp,
            tc.tile_pool(name="work", bufs=3) as work,
            tc.tile_pool(name="mlp", bufs=2) as mlp,
            tc.tile_pool(name="psum", bufs=2, space="PSUM") as psum,
        ):
            nc.gpsimd.load_library(library_config.mlp)

            radial_sb = res.tile([FH, NLOC], f32)
            nc.sync.dma_start(out=radial_sb[:], in_=radial[:])
            invdegw_sb = res.tile([P, n_groups * FH], f32)
            nc.sync.dma_start(out=invdegw_sb[:], in_=invdegw[:])
            ident_sb = res.tile([P, P], f32)
            nc.sync.dma_start(out=ident_sb[:], in_=identt[:])
            w1a_sb = res.tile([FH, HID], f32)
            nc.sync.dma_start(out=w1a_sb[:], in_=w1a[:])
            w1b_sb = res.tile([FH, HID], f32)
            nc.sync.dma_start(out=w1b_sb[:], in_=w1b[:])
            w2_sb = res.tile([HID, F], f32)
            nc.sync.dma_start(out=w2_sb[:], in_=w2[:])
            b1_sb = res.tile([HID, 1], f32)
            nc.sync.dma_start(out=b1_sb[:], in_=b1[:])
            b2_sb = res.tile([1, F], f32)
            nc.sync.dma_start(out=b2_sb[:], in_=b2[:])
            ones_sb = res.tile([1, P], f32)
            nc.vector.memset(ones_sb[:], 1.0)
            cidx_sb = res.tile([P, 4 * NW], i16)
            nc.sync.dma_start(out=cidx_sb[:], in_=cidx[:])

            # ---------------- phase A: chunked gathers + tree reduce ------
            off = 0
            for bi, (q, g0, ks) in enumerate(batches):
                W = idx_cols[bi]
                SK = sum(ks)
                n_idx = P * SK
                it = ixp.tile([P, W], i16, tag="ix")
                nc.sync.dma_start(out=it[:], in_=idxs[:, off : off + W])
                off += W

                G = work.tile([P, SK * FH], f32, tag="G")
                base_row = q * (CHS + 1)
                _dma_gather_raw(
                    nc.gpsimd,
                    G[:].rearrange("p (s e) -> p s e", e=FH),
                    table[base_row:, 0:FH],
                    it[:],
                    n_idx,
                    FH,
                    64,
                )
                S = work.tile([P, len(ks) * FH], f32, tag="S")
                offg = 0
                for gg, K in enumerate(ks):
                    w = K
                    while w > 2:
                        half = w // 2
                        nc.vector.tensor_add(
                            out=G[:, offg * FH : (offg + half) * FH],
                            in0=G[:, offg * FH : (offg + half) * FH],
                            in1=G[
                                :,
                                (offg + w - half) * FH : (offg + w) * FH,
                            ],
                        )
                        w -= half
                    if w == 2:
                        nc.vector.tensor_add(
                            out=S[:, gg * FH : (gg + 1) * FH],
                            in0=G[:, offg * FH : (offg + 1) * FH],
                            in1=G[:, (offg + 1) * FH : (offg + 2) * FH],
                        )
                    else:
                        nc.vector.tensor_copy(
                            S[:, gg * FH : (gg + 1) * FH],
                            G[:, offg * FH : (offg + 1) * FH],
                        )
                    offg += K
                nc.sync.dma_start(
                    out=parts[q][g0 * P : (g0 + len(ks)) * P, 0:FH].rearrange(
                        "(g p) f -> p g f", p=P
                    ),
                    in_=S[:].rearrange("p (g f) -> p g f", f=FH),
                )

            # ------ phase B+C: quarter-combine + MLP (overlapped segs) ----
            hg0 = 0
            for h, HG in enumerate(segs_bc):
                A = mlp.tile([P, HG * FH], f32, tag="Aseg")
                for q in range(4):
                    pq = mlp.tile([P, HG * FH], f32, tag="Pq")
                    _dma_gather_raw(
                        nc.gpsimd,
                        pq[:].rearrange("p (s e) -> p s e", e=FH),
                        parts[q][:, 0:FH],
                        cidx_sb[:, q * NW + hg0 * 8 : q * NW + (hg0 + HG) * 8],
                        HG * P,
                        FH,
                        64,
                    )
                    if q == 0:
                        nc.vector.tensor_copy(A[:], pq[:])
                    else:
                        nc.vector.tensor_add(out=A[:], in0=A[:], in1=pq[:])
                nc.vector.tensor_tensor(
                    out=A[:],
                    in0=A[:],
                    in1=invdegw_sb[:, hg0 * FH : (hg0 + HG) * FH],
                    op=mybir.AluOpType.mult,
                )

                g0h = hg0
                while g0h < hg0 + HG:
                    gs = min(BLOCK_GROUPS, hg0 + HG - g0h)
                    nb = gs * P
                    tr_ps = psum.tile([FH, nb], f32, tag="tr")
                    for cblk in range(gs):
                        g = g0h + cblk
                        nc.tensor.transpose(
                            out=tr_ps[:, cblk * P : (cblk + 1) * P],
                            in_=A[:, (g - hg0) * FH : (g - hg0 + 1) * FH],
                            identity=ident_sb[:],
                        )
                    agg_sb = mlp.tile([FH, nb], f32, tag="agg")
                    nc.scalar.activation(agg_sb[:], tr_ps[:], AF.Copy)

                    h_ps = psum.tile([HID, nb], f32, tag="h")
                    col0 = g0h * P
                    nc.tensor.matmul(
                        h_ps[:],
                        w1a_sb[:],
                        radial_sb[:, col0 : col0 + nb],
                        start=True,
                        stop=False,
                    )
                    nc.tensor.matmul(
                        h_ps[:], w1b_sb[:], agg_sb[:], start=False, stop=True
                    )
                    h_sb = mlp.tile([HID, nb], f32, tag="h_sb")
                    nc.scalar.activation(h_sb[:], h_ps[:], AF.Relu, bias=b1_sb[:, :1])

                    o_ps = psum.tile([P, gs * F], f32, tag="o")
                    for cblk in range(gs):
                        nc.tensor.matmul(
                            o_ps[:, cblk * F : (cblk + 1) * F],
                            ones_sb[:],
                            b2_sb[:],
                            start=True,
                            stop=False,
                        )
                        nc.tensor.matmul(
                            o_ps[:, cblk * F : (cblk + 1) * F],
                            h_sb[:, cblk * P : (cblk + 1) * P],
                            w2_sb[:],
                            start=False,
                            stop=True,
                        )
                    o_sb = mlp.tile([P, gs * F], f32, tag="o_sb")
                    nc.scalar.activation(o_sb[:], o_ps[:], AF.Copy)
                    nc.sync.dma_start(
                        out=out[col0 : col0 + nb, :].rearrange(
                            "(c p) f -> p c f", p=P
                        ),
                        in_=o_sb[:].rearrange("p (c f) -> p c f", f=F),
                    )
                    g0h += gs
                hg0 += HG
    return nc


# ------------------------------------------------------------------ driver


def _run(x, edge_index, W1, b1, W2, b2, trace=False):
    from concourse.bass_utils import run_bass_kernel_spmd

    prep = _host_prep(x, edge_index)
    nc = build_program(prep)
    if not nc.is_finalized():
        nc.finalize()

    W1 = np.ascontiguousarray(W1, np.float32)
    in_maps = []
    for c in range(N_CORES):
        in_maps.append(
            {
                "table": prep["table"],
                "idxs": np.ascontiguousarray(prep["idx_all"][c]),
                "cidx": np.ascontiguousarray(
                    prep["comb_idx"][c].transpose(1, 0, 2).reshape(P, -1)
                ),
                "radial": np.ascontiguousarray(prep["radial_all"][c]),
                "invdegw": np.ascontiguousarray(prep["invdegw_all"][c]),
                "identt": prep["ident"],
                "w1a": np.ascontiguousarray(W1[:FH]),
                "w1b": np.ascontiguousarray(W1[FH:]),
                "w2": np.ascontiguousarray(W2, np.float32),
                "b1": np.ascontiguousarray(b1, np.float32).reshape(HID, 1),
                "b2": np.ascontiguousarray(b2, np.float32).reshape(1, F),
            }
        )
    br = run_bass_kernel_spmd(nc, in_maps, list(range(N_CORES)), trace=trace)

    N = x.shape[0]
    NLOC = prep["NLOC"]
    order = prep["order"]
    result = np.empty((N, F), np.float32)
    r = np.arange(NLOC)
    g = r // P
    p = r % P
    for c in range(N_CORES):
        shard = np.asarray(br.results[c]["out"])
        nid = GROUP * g + P * c + p
        valid = nid < N
        result[order[nid[valid]]] = shard[valid]
    return result, br


def kernel(x, edge_index, W1, b1, W2, b2):
    x = np.ascontiguousarray(np.asarray(x), np.float32)
    edge_index = np.ascontiguousarray(np.asarray(edge_index), np.int32)
    result, _ = _run(
        x,
        edge_index,
        np.asarray(W1),
        np.asarray(b1),
        np.asarray(W2),
        np.asarray(b2),
    )
    return result
